# revision 1
# baseline (speedup 1.0000x reference)
"""MHA Bass kernel for TRN2, 8 NeuronCores.

Sharding: data-parallel on batch (2) x tensor-parallel on heads (4 groups of 4
heads). Core c handles batch c//4 and heads 4*(c%4)..4*(c%4)+3 (columns
m0=256*(c%4)). LayerNorm is folded into the projections as a rank-1
correction (gamma folded into weights on host; mean/var computed on-device
via ones-matmuls); attention computed with transposed scores (S^T) so the
softmax'd P^T feeds the O-matmul directly; softmax denominators ride the
O-matmul as a ones-row (M=65); out-projection partials are ReduceScattered
across each batch group of 4 cores.
"""
import numpy as np

B, LQ, D = 2, 2048, 1024
NHEAD, DHEAD = 16, 64
NC = 8
GPC = 4              # cores per batch group
MPC = 256            # output cols per core
N_DCH = D // 128     # 8 d-chunks
N_TCH = LQ // 128    # 16 token chunks
N_TT = LQ // 512     # 4 token tiles of 512
HPC = 4              # heads per core

_NC_CACHE = [None]


def _build():
    import concourse.bacc as bacc
    import concourse.mybir as mybir
    from concourse import tile

    f32, bf16 = mybir.dt.float32, mybir.dt.bfloat16
    AF = mybir.ActivationFunctionType
    MUL, ADD, SUB = mybir.AluOpType.mult, mybir.AluOpType.add, mybir.AluOpType.subtract

    nc = bacc.Bacc("TRN2", target_bir_lowering=False, debug=False, num_devices=NC)

    xq = nc.dram_tensor("xq", [LQ, D], f32, kind="ExternalInput").ap()
    xk = nc.dram_tensor("xk", [LQ, D], f32, kind="ExternalInput").ap()
    xv = nc.dram_tensor("xv", [LQ, D], f32, kind="ExternalInput").ap()
    wqT = nc.dram_tensor("wqT", [D, MPC], bf16, kind="ExternalInput").ap()
    wkT = nc.dram_tensor("wkT", [D, MPC], bf16, kind="ExternalInput").ap()
    wvT = nc.dram_tensor("wvT", [D, MPC], bf16, kind="ExternalInput").ap()
    wgT = nc.dram_tensor("wgT", [D, MPC], bf16, kind="ExternalInput").ap()
    woT = nc.dram_tensor("woT", [MPC, D], bf16, kind="ExternalInput").ap()
    mucq = nc.dram_tensor("mucq", [1, MPC], bf16, kind="ExternalInput").ap()
    muck = nc.dram_tensor("muck", [1, MPC], bf16, kind="ExternalInput").ap()
    mucv = nc.dram_tensor("mucv", [1, MPC], bf16, kind="ExternalInput").ap()
    mucg = nc.dram_tensor("mucg", [1, MPC], bf16, kind="ExternalInput").ap()
    bq_d = nc.dram_tensor("bq", [MPC], f32, kind="ExternalInput").ap()
    bk_d = nc.dram_tensor("bk", [MPC], f32, kind="ExternalInput").ap()
    bv_d = nc.dram_tensor("bv", [MPC], f32, kind="ExternalInput").ap()
    bg_d = nc.dram_tensor("bgt", [MPC], f32, kind="ExternalInput").ap()
    ident = nc.dram_tensor("ident", [128, 128], bf16, kind="ExternalInput").ap()
    out_d = nc.dram_tensor("out", [MPC, LQ], f32, kind="ExternalOutput").ap()

    EPS = 1024.0 * 1024.0 * 1e-5

    with tile.TileContext(nc) as tc:
        import contextlib
        es = contextlib.ExitStack()
        with es:
            const = es.enter_context(tc.tile_pool(name="const", bufs=1))
            persist = es.enter_context(tc.tile_pool(name="persist", bufs=1))

            ones = const.tile([128, 128], bf16)
            nc.gpsimd.memset(ones[:, :], 1.0)
            eps_t = const.tile([128, 1], f32)
            nc.gpsimd.memset(eps_t[:, :], 1e-5)
            idt = const.tile([128, 128], bf16)
            nc.sync.dma_start(out=idt[:, :], in_=ident[:, :])

            # weights: [128, 8, 256] layouts (d-chunk, cols)
            wts = {}
            for nm, dr in (("q", wqT), ("k", wkT), ("v", wvT), ("g", wgT)):
                t = const.tile([128, N_DCH, MPC], bf16, tag=f"w{nm}")
                for j in range(N_DCH):
                    nc.sync.dma_start(out=t[:, j, :], in_=dr[128 * j:128 * (j + 1), :])
                wts[nm] = t
            wo_t = const.tile([128, 2, D], bf16)
            for mc in range(2):
                nc.sync.dma_start(out=wo_t[:, mc, :], in_=woT[128 * mc:128 * (mc + 1), :])
            mucs = {}
            for nm, dr in (("q", mucq), ("k", muck), ("v", mucv), ("g", mucg)):
                t = const.tile([1, MPC], bf16, tag=f"muc{nm}")
                nc.sync.dma_start(out=t[:, :], in_=dr[:, :])
                mucs[nm] = t
            biases = {}
            for nm, dr in (("q", bq_d), ("k", bk_d), ("v", bv_d), ("g", bg_d)):
                t = const.tile([128, 2], f32, tag=f"b{nm}")
                nc.sync.dma_start(out=t[:, :], in_=dr.rearrange("(c p) -> p c", p=128))
                biases[nm] = t

            # persistent activation outputs
            qhT = persist.tile([128, 2, LQ], bf16, tag="qhT")
            khT = persist.tile([128, 2, LQ], bf16, tag="khT")
            gT = persist.tile([128, 2, LQ], bf16, tag="gT")
            ygT = persist.tile([128, 2, LQ], bf16, tag="ygT")
            vaug = persist.tile([128, N_TCH, HPC, 65], bf16, tag="vaug")
            # contiguous memset; v-transposes later overwrite cols 0:64, col 64 stays 1.0
            nc.gpsimd.memset(vaug[:, :, :, :], 1.0)

            with tc.tile_pool(name="ph1", bufs=2) as ph1, \
                 tc.tile_pool(name="ph1b", bufs=1) as ph1b, \
                 tc.tile_pool(name="stage", bufs=1) as stage, \
                 tc.tile_pool(name="scr", bufs=2) as scr, \
                 tc.tile_pool(name="ps1", bufs=2, space="PSUM") as ps1, \
                 tc.tile_pool(name="pstr", bufs=2, space="PSUM") as pstr:

                def load_T(x_dram):
                    """load [2048,1024] f32 -> bf16 transposed xT [128,(i,j,t)]."""
                    xT = ph1.tile([128, N_TCH, N_DCH, 128], bf16, tag="xT")
                    xr = stage.tile([128, N_TCH, D], bf16, tag="xrow")
                    # 4 cast-DMAs (2.1MB each): out[p, i, c] = x[512q + 128i + p, c]
                    for qq in range(4):
                        nc.gpsimd.dma_start(
                            out=xr[:, 4 * qq:4 * (qq + 1), :],
                            in_=x_dram[512 * qq:512 * (qq + 1), :].rearrange(
                                "(i p) c -> p i c", p=128))
                        for i in range(4 * qq, 4 * qq + 4):
                            nc.sync.dma_start(out=xT[:, i, :, :], in_=xr[:, i, :], transpose=True)
                    return xT

                def rhs_slice(xT, j, tt):
                    return xT[:, 4 * tt:4 * (tt + 1), j, :]

                def stats(xT):
                    """returns r_rep [128,2048] f32 (1/sqrt(n*S2-S1^2+n^2 eps)), mu [1,2048] bf16"""
                    r_rep = ph1b.tile([128, LQ], f32, tag="r_rep")
                    mu = ph1b.tile([1, LQ], bf16, tag="mu")
                    for tt in range(N_TT):
                        sl = slice(512 * tt, 512 * (tt + 1))
                        s1 = ps1.tile([128, 512], f32, tag="s1")
                        s2 = ps1.tile([128, 512], f32, tag="s2")
                        for j in range(N_DCH):
                            nc.tensor.matmul(s1[:, :], ones[:, :], rhs_slice(xT, j, tt),
                                             start=(j == 0), stop=(j == N_DCH - 1))
                        for j in range(N_DCH):
                            sq = scr.tile([128, 512], bf16, tag="sq")
                            nc.vector.tensor_mul(sq[:, :], rhs_slice(xT, j, tt), rhs_slice(xT, j, tt))
                            nc.tensor.matmul(s2[:, :], ones[:, :], sq[:, :],
                                             start=(j == 0), stop=(j == N_DCH - 1))
                        s1s = scr.tile([128, 512], f32, tag="s1s")
                        nc.vector.tensor_copy(s1s[:, :], s1[:, :])
                        t1 = scr.tile([128, 512], f32, tag="t1")
                        nc.vector.tensor_mul(t1[:, :], s1s[:, :], s1s[:, :])
                        t2 = scr.tile([128, 512], f32, tag="t2")
                        nc.vector.scalar_tensor_tensor(t2[:, :], s2[:, :], 1024.0, t1[:, :], MUL, SUB)
                        t3 = scr.tile([128, 512], f32, tag="t3")
                        nc.scalar.activation(t3[:, :], t2[:, :], AF.Sqrt, bias=eps_t[:, :], scale=1.0 / (1024.0 * 1024.0))
                        nc.vector.reciprocal(r_rep[:, sl], t3[:, :])
                        nc.vector.tensor_scalar(mu[0:1, sl], s1s[0:1, :], 1.0 / 1024.0, None, op0=MUL)
                    return r_rep, mu

                def project(xT, wkey, muckey, mu, r_rep, out_t, sigmoid=False):
                    """out_t[:, mc, :] (bf16) = drain((x-mu)@W'^T * r) [*1024 + bias]"""
                    w = wts[wkey]
                    mc_t = mucs[muckey]
                    bias = biases[muckey if not sigmoid else "g"]
                    for mc in range(2):
                        for tt in range(N_TT):
                            sl = slice(512 * tt, 512 * (tt + 1))
                            pp = ps1.tile([128, 512], f32, tag="pp")
                            for j in range(N_DCH):
                                nc.tensor.matmul(pp[:, :], w[:, j, 128 * mc:128 * (mc + 1)],
                                                 rhs_slice(xT, j, tt), start=(j == 0), stop=False)
                            nc.tensor.matmul(pp[:, :], mc_t[:, 128 * mc:128 * (mc + 1)],
                                             mu[0:1, sl], start=False, stop=True)
                            if sigmoid:
                                tmp = scr.tile([128, 512], f32, tag="ptmp")
                                nc.vector.tensor_mul(tmp[:, :], pp[:, :], r_rep[:, sl])
                                nc.scalar.activation(out_t[:, mc, sl], tmp[:, :], AF.Sigmoid,
                                                     bias=bias[:, mc:mc + 1], scale=1.0)
                            else:
                                nc.vector.scalar_tensor_tensor(
                                    out_t[:, mc, sl], pp[:, :], bias[:, mc:mc + 1],
                                    r_rep[:, sl], ADD, MUL)

                # ---- q ----
                xT = load_T(xq)
                r_rep, mu = stats(xT)
                project(xT, "q", "q", mu, r_rep, qhT)
                project(xT, "g", "g", mu, r_rep, gT, sigmoid=True)
                # ---- k ----
                xT = load_T(xk)
                r_rep, mu = stats(xT)
                project(xT, "k", "k", mu, r_rep, khT)
                # ---- v ----
                xT = load_T(xv)
                r_rep, mu = stats(xT)
                vhT = ph1b.tile([128, 2, LQ], bf16, tag="vhT")
                project(xT, "v", "v", mu, r_rep, vhT)
                # transpose vhT -> vaug
                for mc in range(2):
                    for s in range(N_TCH):
                        pt = pstr.tile([128, 128], bf16, tag="ptr")
                        nc.tensor.transpose(pt[:, :], vhT[:, mc, 128 * s:128 * (s + 1)], idt[:, :])
                        nc.vector.tensor_copy(vaug[:, s, 2 * mc, 0:64], pt[:, 0:64])
                        nc.vector.tensor_copy(vaug[:, s, 2 * mc + 1, 0:64], pt[:, 64:128])

            # ---- attention: head pairs; 4 independent (head x tt) chains of
            # [128,512] ST -> exp -> O for fine-grained PE/ACT pipelining
            with tc.tile_pool(name="att", bufs=2) as att, \
                 tc.tile_pool(name="ps_st", bufs=1, space="PSUM") as ps_st, \
                 tc.tile_pool(name="ps_o", bufs=1, space="PSUM") as ps_o:
                for hp in range(2):
                    kc = hp
                    for th in range(2):
                        t0 = 1024 * th
                        o_ps = {}
                        for hb in range(2):
                            for tt in range(2):
                                o_ps[hb, tt] = ps_o.tile([65, 512], f32, name=f"o{hb}{tt}", tag=f"o{hb}{tt}")
                        for s in range(N_TCH):
                            for hb in range(2):
                                r0 = 64 * hb
                                for tt in range(2):
                                    sl_t = slice(t0 + 512 * tt, t0 + 512 * (tt + 1))
                                    st = ps_st.tile([128, 512], f32, tag=f"st{hb}{tt}")
                                    nc.tensor.matmul(st[:, :],
                                                     khT[r0:r0 + 64, kc, 128 * s:128 * (s + 1)],
                                                     qhT[r0:r0 + 64, kc, sl_t],
                                                     start=True, stop=True)
                                    pt = att.tile([128, 512], bf16, tag=f"pt{hb}{tt}")
                                    nc.scalar.activation(pt[:, :], st[:, :], AF.Exp, scale=0.125)
                                    nc.tensor.matmul(o_ps[hb, tt][:, :],
                                                     vaug[:, s, 2 * hp + hb, :], pt[:, :],
                                                     start=(s == 0), stop=(s == N_TCH - 1))
                        for hb in range(2):
                            r0 = 64 * hb
                            for tt in range(2):
                                o_p = o_ps[hb, tt]
                                sl_y = slice(t0 + 512 * tt, t0 + 512 * (tt + 1))
                                li_f = att.tile([65, 512], f32, tag="lif")
                                nc.vector.reciprocal(li_f[64:65, :], o_p[64:65, :])
                                li_b = att.tile([65, 512], bf16, tag="lib")
                                nc.vector.tensor_copy(li_b[64:65, :], li_f[64:65, :])
                                bc = ps_st.tile([64, 512], f32, tag=f"st{hb}{tt}")
                                nc.tensor.matmul(bc[:, :], ones[64:65, 0:64],
                                                 li_b[64:65, :], start=True, stop=True)
                                bcs = att.tile([64, 512], f32, tag="bcs")
                                nc.vector.tensor_copy(bcs[:, :], bc[:, :])
                                tmp2 = att.tile([128, 512], f32, tag="tmp2")
                                nc.vector.tensor_mul(tmp2[0:64, :], o_p[0:64, :], bcs[:, :])
                                if r0 != 0:
                                    # partition shift 0->64 via SBUF->SBUF DMA
                                    nc.gpsimd.dma_start(out=tmp2[64:128, :], in_=tmp2[0:64, :])
                                nc.vector.tensor_mul(ygT[r0:r0 + 64, kc, sl_y],
                                                     tmp2[r0:r0 + 64, :],
                                                     gT[r0:r0 + 64, kc, sl_y])

            # ---- out-proj + reduce-scatter ----
            with tc.tile_pool(name="po", bufs=2, space="PSUM") as po_p, \
                 tc.tile_pool(name="od", bufs=4) as od_p, \
                 tc.tile_pool(name="dram", bufs=1, space="DRAM") as dram_p:
                outb = dram_p.tile([D, LQ], f32, tag="outb")
                outrs = dram_p.tile([MPC, LQ], f32, tag="outrs")
                for half in range(4):
                    for nk in range(2 * half, 2 * half + 2):
                        for tt in range(N_TT):
                            po = po_p.tile([128, 512], f32, tag="po")
                            for mc in range(2):
                                nc.tensor.matmul(po[:, :], wo_t[:, mc, 128 * nk:128 * (nk + 1)],
                                                 ygT[:, mc, 512 * tt:512 * (tt + 1)],
                                                 start=(mc == 0), stop=(mc == 1))
                            ot = od_p.tile([128, 512], f32, tag="ot")
                            nc.vector.tensor_copy(ot[:, :], po[:, :])
                            nc.sync.dma_start(
                                out=outb[128 * nk:128 * (nk + 1), 512 * tt:512 * (tt + 1)],
                                in_=ot[:, :])
                    nc.gpsimd.collective_compute(
                        "ReduceScatter", ADD,
                        replica_groups=[[0, 1, 2, 3], [4, 5, 6, 7]],
                        ins=[outb[256 * half:256 * (half + 1), :].opt()],
                        outs=[outrs[64 * half:64 * (half + 1), :].opt()],
                    )
                for ch in range(2):
                    ob = od_p.tile([128, LQ], f32, tag="ob")
                    nc.sync.dma_start(out=ob[:, :], in_=outrs[128 * ch:128 * (ch + 1), :])
                    nc.sync.dma_start(out=out_d[128 * ch:128 * (ch + 1), :], in_=ob[:, :])

    nc.compile()
    return nc


def kernel(q, k, v, qln_g, qln_b, kvln_g, kvln_b, Wq, Wk, Wv, Wg, bg, Wo):
    import concourse.mybir as mybir
    from concourse import bass_utils

    bf16 = mybir.dt.np(mybir.dt.bfloat16)
    q = np.asarray(q, np.float32)
    k = np.asarray(k, np.float32)
    v = np.asarray(v, np.float32)
    qln_g = np.asarray(qln_g, np.float32)
    qln_b = np.asarray(qln_b, np.float32)
    kvln_g = np.asarray(kvln_g, np.float32)
    kvln_b = np.asarray(kvln_b, np.float32)
    Wq, Wk, Wv = np.asarray(Wq, np.float32), np.asarray(Wk, np.float32), np.asarray(Wv, np.float32)
    Wg, Wo = np.asarray(Wg, np.float32), np.asarray(Wo, np.float32)
    bg = np.asarray(bg, np.float32)

    # fold LN gamma into weights; beta into bias vectors
    Wqp, Wgp = Wq * qln_g[None, :], Wg * qln_g[None, :]
    Wkp, Wvp = Wk * kvln_g[None, :], Wv * kvln_g[None, :]
    bq_f, bk_f, bv_f = Wq @ qln_b, Wk @ kvln_b, Wv @ kvln_b
    bg_f = Wg @ qln_b + bg
    idm = np.eye(128, dtype=np.float32)

    if _NC_CACHE[0] is None:
        _NC_CACHE[0] = _build()
    nc = _NC_CACHE[0]

    in_maps = []
    for c in range(NC):
        beta, g = c // GPC, c % GPC
        m0 = MPC * g
        sl = slice(m0, m0 + MPC)
        in_maps.append({
            "xq": q[beta], "xk": k[beta], "xv": v[beta],
            "wqT": Wqp[sl, :].T.astype(bf16), "wkT": Wkp[sl, :].T.astype(bf16),
            "wvT": Wvp[sl, :].T.astype(bf16), "wgT": Wgp[sl, :].T.astype(bf16),
            "woT": Wo[:, sl].T.astype(bf16),
            "mucq": -Wqp[sl, :].sum(1)[None, :].astype(bf16),
            "muck": -Wkp[sl, :].sum(1)[None, :].astype(bf16),
            "mucv": -Wvp[sl, :].sum(1)[None, :].astype(bf16),
            "mucg": -Wgp[sl, :].sum(1)[None, :].astype(bf16),
            "bq": bq_f[sl], "bk": bk_f[sl], "bv": bv_f[sl], "bgt": bg_f[sl],
            "ident": idm.astype(bf16),
        })
    global _last_in_maps
    _last_in_maps = in_maps
    res = bass_utils.run_bass_kernel_spmd(nc, in_maps, core_ids=list(range(NC)))
    out = np.empty((B, LQ, D), np.float32)
    for beta in range(B):
        rows = np.empty((D, LQ), np.float32)
        for qtr in range(4):
            for g in range(GPC):
                rows[256 * qtr + 64 * g:256 * qtr + 64 * (g + 1)] = \
                    res.results[GPC * beta + g]["out"][64 * qtr:64 * (qtr + 1)]
        out[beta] = rows.T
    return out



# revision 24
# speedup vs baseline: 1.7347x; 1.7347x over previous
"""MHA Bass kernel for TRN2, 8 NeuronCores.

Sharding: data-parallel on batch (2) x tensor-parallel on heads (4 groups of 4
heads). Core c handles batch c//4 and heads 4*(c%4)..4*(c%4)+3 (columns
m0=256*(c%4)).

v2 design vs baseline:
- Inputs pre-cast to bf16 and pre-transposed on host -> xT arrives [1024,2048]
  bf16; direct chunked loads, no on-device cast or SBUF transposes.
- LayerNorm folded into projections (gamma on host, mean/var on device via
  ones-matmuls; rstd via ACT Sqrt + DVE reciprocal_approx_fast).
- v-heads transposed into natural layout via DMA-transpose (no PE transposes).
- Attention with transposed scores (S^T); exp batched N=1024 per ACT instr
  (2 PSUM banks); softmax denominators ride the O-matmul as a ones-column.
- q/g projections interleaved with attention per 512-token tile; out-proj and
  per-tile ReduceScatter pipelined under the next tile's attention.
"""
import numpy as np

B, LQ, D = 2, 2048, 1024
NHEAD, DHEAD = 16, 64
NC = 8
GPC = 4              # cores per batch group
MPC = 256            # output cols per core
N_DCH = D // 128     # 8 d-chunks
N_TT = LQ // 512     # 4 token tiles of 512
N_SCH = LQ // 128    # 16 key chunks

_NC_CACHE = [None]
DEBUG_DUMPS = False


def _build():
    import concourse.bacc as bacc
    import concourse.mybir as mybir
    from concourse import tile

    f32, bf16 = mybir.dt.float32, mybir.dt.bfloat16
    AF = mybir.ActivationFunctionType
    MUL, ADD, SUB = mybir.AluOpType.mult, mybir.AluOpType.add, mybir.AluOpType.subtract

    nc = bacc.Bacc("TRN2", target_bir_lowering=False, debug=False, num_devices=NC)

    xqT = nc.dram_tensor("xqT", [D, LQ], bf16, kind="ExternalInput").ap()
    xkT = nc.dram_tensor("xkT", [D, LQ], bf16, kind="ExternalInput").ap()
    xvT = nc.dram_tensor("xvT", [D, LQ], bf16, kind="ExternalInput").ap()
    wqT = nc.dram_tensor("wqT", [D, MPC], bf16, kind="ExternalInput").ap()
    wkT = nc.dram_tensor("wkT", [D, MPC], bf16, kind="ExternalInput").ap()
    wvT = nc.dram_tensor("wvT", [D, MPC], bf16, kind="ExternalInput").ap()
    wgT = nc.dram_tensor("wgT", [D, MPC], bf16, kind="ExternalInput").ap()
    woT = nc.dram_tensor("woT", [MPC, D], bf16, kind="ExternalInput").ap()
    mucq = nc.dram_tensor("mucq", [1, MPC], bf16, kind="ExternalInput").ap()
    muck = nc.dram_tensor("muck", [1, MPC], bf16, kind="ExternalInput").ap()
    mucv = nc.dram_tensor("mucv", [1, MPC], bf16, kind="ExternalInput").ap()
    mucg = nc.dram_tensor("mucg", [1, MPC], bf16, kind="ExternalInput").ap()
    bq_d = nc.dram_tensor("bq", [MPC], f32, kind="ExternalInput").ap()
    bk_d = nc.dram_tensor("bk", [MPC], f32, kind="ExternalInput").ap()
    bv_d = nc.dram_tensor("bv", [MPC], f32, kind="ExternalInput").ap()
    bg_d = nc.dram_tensor("bgt", [MPC], f32, kind="ExternalInput").ap()
    out_d = nc.dram_tensor("out", [MPC, LQ], bf16, kind="ExternalOutput").ap()
    dbg = {}
    if DEBUG_DUMPS:
        for nm, shp in (("d_khT", [128, 2, LQ]), ("d_qhT", [128, 2, LQ]),
                        ("d_gT", [128, 2, LQ]), ("d_ygT", [128, 2, LQ]),
                        ("d_vaug", [128, N_SCH, 4, 65])):
            dbg[nm] = nc.dram_tensor(nm, shp, mybir.dt.bfloat16,
                                     kind="ExternalOutput").ap()
        dbg["d_rq"] = nc.dram_tensor("d_rq", [128, LQ], f32, kind="ExternalOutput").ap()
        dbg["d_outb"] = nc.dram_tensor("d_outb", [N_TT, D, 512], mybir.dt.bfloat16,
                                       kind="ExternalOutput").ap()

    with tile.TileContext(nc) as tc:
        import contextlib
        es = contextlib.ExitStack()
        with es:
            const = es.enter_context(tc.tile_pool(name="const", bufs=1))
            persist = es.enter_context(tc.tile_pool(name="persist", bufs=1))

            ones = const.tile([128, 128], bf16)
            nc.gpsimd.memset(ones[:, :], 1.0)
            eps_t = const.tile([128, 1], f32)
            nc.gpsimd.memset(eps_t[:, :], 1e-5)

            # weights: [128, 8, 256] layouts (d-chunk, cols)
            wts = {}
            for nm, dr in (("q", wqT), ("k", wkT), ("v", wvT), ("g", wgT)):
                t = const.tile([128, N_DCH, MPC], bf16, tag=f"w{nm}")
                nc.sync.dma_start(out=t[:, :, :],
                                  in_=dr.rearrange("(c p) m -> p c m", p=128))
                wts[nm] = t
            wo_t = const.tile([128, 2, D], bf16)
            nc.sync.dma_start(out=wo_t[:, :, :],
                              in_=woT.rearrange("(c p) d -> p c d", p=128))
            mucs = {}
            for nm, dr in (("q", mucq), ("k", muck), ("v", mucv), ("g", mucg)):
                t = const.tile([1, MPC], bf16, tag=f"muc{nm}")
                nc.sync.dma_start(out=t[:, :], in_=dr[:, :])
                mucs[nm] = t
            biases = {}
            for nm, dr in (("q", bq_d), ("k", bk_d), ("v", bv_d), ("g", bg_d)):
                t = const.tile([128, 2], f32, tag=f"b{nm}")
                nc.sync.dma_start(out=t[:, :], in_=dr.rearrange("(c p) -> p c", p=128))
                biases[nm] = t

            # persistent activation outputs
            qhT = persist.tile([128, 2, LQ], bf16, tag="qhT")
            khT = persist.tile([128, 2, LQ], bf16, tag="khT")
            gT = persist.tile([128, 2, LQ], bf16, tag="gT")
            ygT = persist.tile([128, 2, LQ], bf16, tag="ygT")
            vaug = persist.tile([128, N_SCH, 4, 65], bf16, tag="vaug")
            # contiguous memset; v-transposes later overwrite cols 0:64, col 64 stays 1.0
            nc.gpsimd.memset(vaug[:, :, :, :], 1.0)

            xpool = es.enter_context(tc.tile_pool(name="xp", bufs=2))
            statp = es.enter_context(tc.tile_pool(name="stat", bufs=2))
            vhp = es.enter_context(tc.tile_pool(name="vhp", bufs=1))
            scr = es.enter_context(tc.tile_pool(name="scr", bufs=2))
            # shared [128,512] f32 PSUM tiles: proj pp, out-proj po, bcast bc
            ps_w = es.enter_context(tc.tile_pool(name="ps_w", bufs=2, space="PSUM"))

            def load_x(dr):
                """[1024, 2048] bf16 DRAM -> xT [128, 8, 2048] (per-chunk DMAs)."""
                xT = xpool.tile([128, N_DCH, LQ], bf16, tag="xT")
                for j in range(N_DCH):
                    nc.sync.dma_start(out=xT[:, j, :],
                                      in_=dr[128 * j:128 * (j + 1), :])
                return xT

            def stats(xT, ps_s):
                """r_rep [128,2048] f32 = rstd (broadcast); mu [1,2048] bf16."""
                r_rep = statp.tile([128, LQ], f32, tag="r_rep")
                mu = statp.tile([1, LQ], bf16, tag="mu")
                for tt in range(N_TT):
                    sl = slice(512 * tt, 512 * (tt + 1))
                    s1 = ps_s.tile([128, 512], f32, tag="s1")
                    s2 = ps_s.tile([128, 512], f32, tag="s2")
                    for j in range(N_DCH):
                        nc.tensor.matmul(s1[:, :], ones[:, :], xT[:, j, sl],
                                         start=(j == 0), stop=(j == N_DCH - 1))
                    for j in range(N_DCH):
                        sq = scr.tile([128, 512], bf16, tag="sq")
                        nc.vector.tensor_mul(sq[:, :], xT[:, j, sl], xT[:, j, sl])
                        nc.tensor.matmul(s2[:, :], ones[:, :], sq[:, :],
                                         start=(j == 0), stop=(j == N_DCH - 1))
                    s1s = scr.tile([128, 512], f32, tag="s1s")
                    nc.vector.tensor_copy(s1s[:, :], s1[:, :])
                    t1 = scr.tile([128, 512], f32, tag="t1")
                    nc.vector.tensor_mul(t1[:, :], s1s[:, :], s1s[:, :])
                    t2 = scr.tile([128, 512], f32, tag="t2")
                    nc.vector.scalar_tensor_tensor(t2[:, :], s2[:, :], 1024.0, t1[:, :], MUL, SUB)
                    t3 = scr.tile([128, 512], f32, tag="t3")
                    nc.scalar.activation(t3[:, :], t2[:, :], AF.Sqrt,
                                         bias=eps_t[:, :], scale=1.0 / (1024.0 * 1024.0))
                    nc.vector.reciprocal_approx_fast(r_rep[:, sl], t3[:, :])
                    nc.vector.tensor_scalar(mu[0:1, sl], s1s[0:1, :], 1.0 / 1024.0, None, op0=MUL)
                return r_rep, mu

            def project_tt(xT, wkey, muckey, mu, r_rep, out_t, tt, sigmoid=False):
                """out_t[:, mc, tt-slice] = ((x-mu)@W'^T)*rstd [+bias / sigmoid]"""
                w = wts[wkey]
                mc_t = mucs[muckey]
                bias = biases[muckey]
                sl = slice(512 * tt, 512 * (tt + 1))
                for mc in range(2):
                    pp = ps_w.tile([128, 512], f32, tag="w")
                    for j in range(N_DCH):
                        nc.tensor.matmul(pp[:, :], w[:, j, 128 * mc:128 * (mc + 1)],
                                         xT[:, j, sl], start=(j == 0), stop=False)
                    nc.tensor.matmul(pp[:, :], mc_t[:, 128 * mc:128 * (mc + 1)],
                                     mu[0:1, sl], start=False, stop=True)
                    if sigmoid:
                        # sigmoid(x+b) = 0.5*tanh(0.5*(x+b)) + 0.5 -- tanh shares
                        # the exp table set (no ACT table thrash in attention)
                        tmp = scr.tile([128, 512], f32, tag="ptmp")
                        nc.vector.tensor_mul(tmp[:, :], pp[:, :], r_rep[:, sl])
                        tnh = scr.tile([128, 512], f32, tag="tnh")
                        nc.scalar.activation(tnh[:, :], tmp[:, :], AF.Tanh,
                                             bias=bias[:, mc:mc + 1], scale=0.5)
                        nc.vector.tensor_scalar(out_t[:, mc, sl], tnh[:, :],
                                                0.5, 0.5, op0=MUL, op1=ADD)
                    else:
                        nc.vector.scalar_tensor_tensor(
                            out_t[:, mc, sl], pp[:, :], bias[:, mc:mc + 1],
                            r_rep[:, sl], ADD, MUL)

            with tc.tile_pool(name="ps_s", bufs=2, space="PSUM") as ps_s:
                # ---- k ----
                xT = load_x(xkT)
                r_rep, mu = stats(xT, ps_s)
                for tt in range(N_TT):
                    project_tt(xT, "k", "k", mu, r_rep, khT, tt)
                # ---- v ----
                xTv = load_x(xvT)
                r_rep, mu = stats(xTv, ps_s)
                vhT = vhp.tile([128, 2, LQ], bf16, tag="vhT")
                for tt in range(N_TT):
                    project_tt(xTv, "v", "v", mu, r_rep, vhT, tt)
                # v -> natural layout [k-token, dh]: DMA transpose needs a
                # contiguous output, so bounce via scratch then strided DVE copy
                with tc.tile_pool(name="vtrp", bufs=1) as vtrp:
                    for mc in range(2):
                        for hb in range(2):
                            vtr = vtrp.tile([128, N_SCH, 64], bf16, tag="vtr")
                            nc.sync.dma_start_transpose(
                                out=vtr[:, :, :],
                                in_=vhT[64 * hb:64 * (hb + 1), mc, :])
                            nc.vector.tensor_copy(vaug[:, :, 2 * mc + hb, 0:64],
                                                  vtr[:, :, :])
                # ---- q (stats only; projections interleaved with attention) ----
                xTq = load_x(xqT)
                r_q, mu_q = stats(xTq, ps_s)

            # ---- attention + out-proj + RS, token-tile outer ----
            att = es.enter_context(tc.tile_pool(name="att", bufs=2))
            attn = es.enter_context(tc.tile_pool(name="attn", bufs=2))
            ps_st = es.enter_context(tc.tile_pool(name="ps_st", bufs=2, space="PSUM"))
            ps_o = es.enter_context(tc.tile_pool(name="ps_o", bufs=1, space="PSUM"))
            od = es.enter_context(tc.tile_pool(name="od", bufs=2))
            dram_p = es.enter_context(tc.tile_pool(name="dram", bufs=1, space="DRAM"))
            outb = dram_p.tile([N_TT, D, 512], bf16, tag="outb")
            outrs = dram_p.tile([N_TT, MPC, 512], bf16, tag="outrs")

            for th in range(N_TT):
                slth = slice(512 * th, 512 * (th + 1))
                # project this tile's q and g
                project_tt(xTq, "q", "q", mu_q, r_q, qhT, th)
                project_tt(xTq, "g", "g", mu_q, r_q, gT, th, sigmoid=True)
                for hp in range(2):
                    o4 = ps_o.tile([65, 2, 512], f32, tag="o4")
                    for s in range(N_SCH):
                        st2 = ps_st.tile([128, 2, 512], f32, tag="st2")
                        for hb in range(2):
                            nc.tensor.matmul(
                                st2[:, hb, :],
                                khT[64 * hb:64 * (hb + 1), hp, 128 * s:128 * (s + 1)],
                                qhT[64 * hb:64 * (hb + 1), hp, slth],
                                start=True, stop=True)
                        pt = att.tile([128, 2, 512], bf16, tag="pt")
                        nc.scalar.activation(pt[:, :, :], st2[:, :, :], AF.Exp, scale=0.125)
                        for hb in range(2):
                            nc.tensor.matmul(o4[:, hb, :],
                                             vaug[:, s, 2 * hp + hb, :], pt[:, hb, :],
                                             start=(s == 0), stop=(s == N_SCH - 1))
                    # normalize + gate: broadcast denominator via K=1 matmul,
                    # reciprocal at base partition 0 (approx_fast is broken at
                    # nonzero base partitions on HW)
                    for hb in range(2):
                        r0 = 64 * hb
                        li_b = attn.tile([65, 512], bf16, tag="lib")
                        nc.vector.tensor_copy(li_b[64:65, :], o4[64:65, hb, :])
                        bc = ps_w.tile([128, 512], f32, tag="w")
                        nc.tensor.matmul(bc[0:64, :], ones[64:65, 0:64],
                                         li_b[64:65, :], start=True, stop=True)
                        bcs = attn.tile([64, 512], f32, tag="bcs")
                        nc.vector.tensor_copy(bcs[:, :], bc[0:64, :])
                        bcr = attn.tile([64, 512], f32, tag="bcr")
                        nc.vector.reciprocal_approx_fast(bcr[:, :], bcs[:, :])
                        tmp2 = attn.tile([128, 512], f32, tag="tmp2")
                        nc.vector.tensor_mul(tmp2[0:64, :], o4[0:64, hb, :], bcr[:, :])
                        if r0 != 0:
                            # partition shift 0->64 via SBUF->SBUF DMA
                            nc.gpsimd.dma_start(out=tmp2[64:128, :], in_=tmp2[0:64, :])
                        nc.vector.tensor_mul(ygT[r0:r0 + 64, hp, slth],
                                             tmp2[r0:r0 + 64, :],
                                             gT[r0:r0 + 64, hp, slth])
                # out-proj for this token tile
                for nk in range(N_DCH):
                    po = ps_w.tile([128, 512], f32, tag="w")
                    for mc in range(2):
                        nc.tensor.matmul(po[:, :], wo_t[:, mc, 128 * nk:128 * (nk + 1)],
                                         ygT[:, mc, slth],
                                         start=(mc == 0), stop=(mc == 1))
                    ot = od.tile([128, 512], bf16, tag="ot")
                    nc.vector.tensor_copy(ot[:, :], po[:, :])
                    nc.sync.dma_start(out=outb[th, 128 * nk:128 * (nk + 1), :],
                                      in_=ot[:, :])
                nc.gpsimd.collective_compute(
                    "ReduceScatter", ADD,
                    replica_groups=[[0, 1, 2, 3], [4, 5, 6, 7]],
                    ins=[outb[th, :, :].opt()],
                    outs=[outrs[th, :, :].opt()],
                )
                nc.sync.dma_start(out=out_d[:, 512 * th:512 * (th + 1)],
                                  in_=outrs[th, :, :])
            if DEBUG_DUMPS:
                nc.sync.dma_start(out=dbg["d_khT"][:, :, :], in_=khT[:, :, :])
                nc.sync.dma_start(out=dbg["d_qhT"][:, :, :], in_=qhT[:, :, :])
                nc.sync.dma_start(out=dbg["d_gT"][:, :, :], in_=gT[:, :, :])
                nc.sync.dma_start(out=dbg["d_ygT"][:, :, :], in_=ygT[:, :, :])
                nc.sync.dma_start(out=dbg["d_vaug"][:, :, :, :], in_=vaug[:, :, :, :])
                nc.sync.dma_start(out=dbg["d_rq"][:, :], in_=r_q[:, :])
                nc.sync.dma_start(out=dbg["d_outb"][:, :, :], in_=outb[:, :, :])

    nc.compile()
    return nc


def kernel(q, k, v, qln_g, qln_b, kvln_g, kvln_b, Wq, Wk, Wv, Wg, bg, Wo):
    import concourse.mybir as mybir
    from concourse import bass_utils

    bf16 = mybir.dt.np(mybir.dt.bfloat16)
    q = np.asarray(q, np.float32)
    k = np.asarray(k, np.float32)
    v = np.asarray(v, np.float32)
    qln_g = np.asarray(qln_g, np.float32)
    qln_b = np.asarray(qln_b, np.float32)
    kvln_g = np.asarray(kvln_g, np.float32)
    kvln_b = np.asarray(kvln_b, np.float32)
    Wq, Wk, Wv = np.asarray(Wq, np.float32), np.asarray(Wk, np.float32), np.asarray(Wv, np.float32)
    Wg, Wo = np.asarray(Wg, np.float32), np.asarray(Wo, np.float32)
    bg = np.asarray(bg, np.float32)

    # fold LN gamma into weights; beta into bias vectors
    Wqp, Wgp = Wq * qln_g[None, :], Wg * qln_g[None, :]
    Wkp, Wvp = Wk * kvln_g[None, :], Wv * kvln_g[None, :]
    bq_f, bk_f, bv_f = Wq @ qln_b, Wk @ kvln_b, Wv @ kvln_b
    bg_f = (Wg @ qln_b + bg) * 0.5  # pre-halved for the tanh-form sigmoid

    if _NC_CACHE[0] is None:
        _NC_CACHE[0] = _build()
    nc = _NC_CACHE[0]

    # pre-transposed bf16 inputs, shared per batch
    xT = {}
    for beta in range(B):
        xT[("q", beta)] = np.ascontiguousarray(q[beta].T).astype(bf16)
        xT[("k", beta)] = np.ascontiguousarray(k[beta].T).astype(bf16)
        xT[("v", beta)] = np.ascontiguousarray(v[beta].T).astype(bf16)

    in_maps = []
    for c in range(NC):
        beta, g = c // GPC, c % GPC
        sl = slice(MPC * g, MPC * (g + 1))
        in_maps.append({
            "xqT": xT[("q", beta)], "xkT": xT[("k", beta)], "xvT": xT[("v", beta)],
            "wqT": Wqp[sl, :].T.astype(bf16), "wkT": Wkp[sl, :].T.astype(bf16),
            "wvT": Wvp[sl, :].T.astype(bf16), "wgT": Wgp[sl, :].T.astype(bf16),
            "woT": Wo[:, sl].T.astype(bf16),
            "mucq": -Wqp[sl, :].sum(1)[None, :].astype(bf16),
            "muck": -Wkp[sl, :].sum(1)[None, :].astype(bf16),
            "mucv": -Wvp[sl, :].sum(1)[None, :].astype(bf16),
            "mucg": -Wgp[sl, :].sum(1)[None, :].astype(bf16),
            "bq": bq_f[sl], "bk": bk_f[sl], "bv": bv_f[sl], "bgt": bg_f[sl],
        })
    global _last_in_maps
    _last_in_maps = in_maps
    res = bass_utils.run_bass_kernel_spmd(nc, in_maps, core_ids=list(range(NC)))
    out = np.empty((B, LQ, D), np.float32)
    for beta in range(B):
        for g in range(GPC):
            out[beta, :, MPC * g:MPC * (g + 1)] = \
                res.results[GPC * beta + g]["out"].astype(np.float32).T
    return out
